# revision 13
# baseline (speedup 1.0000x reference)
"""Trainium2 Bass kernel for MultiHeadAttention with MLP (additive) scores.

Reference computation (per batch b):
  qh = q @ Wq + bq; kh = k @ Wk; vh = v @ Wv           (heads x hdim = 16 x 64)
  s[i,j,n] = qh[i,n,:]@w1 + kh[j,n,:]@w2               (folded: q @ Wq1 + bq1, k @ Wk2)
  attn = softmax_j(tanh(s)); ctx = attn @ vh
  out = LayerNorm(ctx @ Wp + bp) * gamma + beta

Sharding: core c = 2*b + ihalf handles batch b = c//2, query rows
[ihalf*512, ihalf*512+512), all 16 heads. No collectives.

On-chip schedule per core (all big matmuls in float32r, 1 cyc/row):
  sq[16,512] = Wq1^T @ qT (+bq1), sk[16,1024] = Wk2^T @ kT
  vh[j, h*65:(h+1)*65] = [v @ Wv heads | ones-col]     (denominator trick)
  per head-pair: s = sk (x) 1 + 1 (x) sq   (rank-2 PE outer into PSUM pair)
                 t = Tanh(s); g = Exp(t)               (ACT, same table set)
                 ctx_h[65,512] += vh_aug[jt]^T @ g     (row 64 = softmax denom)
  normalize: rb = 1/(ones (x) den); ctx *= rb          (DVE recip-approx + TT)
  out[i,:] = ctx_all^T @ Wp + 1 (x) bp; LayerNorm on DVE/ACT; DMA out.
"""
import ml_dtypes
import numpy as np

BF = ml_dtypes.bfloat16

import concourse.bacc as bacc
import concourse.mybir as mybir
from concourse.tile import TileContext
from concourse.bass_utils import run_bass_kernel_spmd

F32 = mybir.dt.float32
BF16 = mybir.dt.bfloat16
AF = mybir.ActivationFunctionType
ALU = mybir.AluOpType

B, L, E, HEADS, HDIM, OUT = 4, 1024, 1024, 16, 64, 1024
N_CORES = 8
IH = L // 2          # 512 query rows per core
JT = L // 128        # 8 key tiles
ET = E // 128        # 8 embed tiles
LN_EPS = 1e-6

_CACHE = {}


def _build(apply_gamma_beta: bool):
    nc = bacc.Bacc("TRN2", target_bir_lowering=False, debug=False,
                   num_devices=N_CORES)

    def din(name, shape, dt=BF16):
        return nc.dram_tensor(name, shape, dt, kind="ExternalInput").ap()

    qT = din("qT", [E, IH])          # q-half transposed  [e, i]
    kT = din("kT", [E, L])           # [e, j]
    vT = din("vT", [E, L])           # [e, j]
    wv = din("wv", [E, OUT])         # [e, nd]
    wp = din("wp", [HEADS * HDIM, OUT])   # [nd, o]
    wq1 = din("wq1", [E, HEADS])
    wk2 = din("wk2", [E, HEADS])
    bq1 = din("bq1", [HEADS, 1], F32)
    bp_r = din("bp_r", [1, OUT])
    ones128 = din("ones128", [128, 128])
    if apply_gamma_beta:
        gma = din("gma", [128, OUT], F32)
        bta = din("bta", [128, OUT], F32)
    out_d = nc.dram_tensor("out", [IH, OUT], F32, kind="ExternalOutput").ap()

    with TileContext(nc) as tc:
        with (
            tc.tile_pool(name="big", bufs=1) as big,       # long-lived SBUF
            tc.tile_pool(name="work", bufs=1) as work,     # pipelined SBUF
            tc.tile_pool(name="ps", bufs=1, space="PSUM") as ps,
        ):
            # ---- static loads -------------------------------------------
            kt_sb = [big.tile([128, L], BF16, tag="w8", bufs=8, name=f"kt{t}")
                     for t in range(ET)]
            vt_sb = [big.tile([128, L], BF16, tag="v8", bufs=8, name=f"vt{t}")
                     for t in range(ET)]
            wv_sb = [big.tile([128, OUT], BF16, tag="wv8", bufs=8, name=f"wv{t}")
                     for t in range(ET)]
            qt_sb = [big.tile([128, IH], BF16, tag="q8", bufs=8, name=f"qt{t}")
                     for t in range(ET)]
            wq1_sb = [big.tile([128, HEADS], BF16, tag="wq1", bufs=8, name=f"wq1{t}")
                      for t in range(ET)]
            wk2_sb = [big.tile([128, HEADS], BF16, tag="wk2", bufs=8, name=f"wk2{t}")
                      for t in range(ET)]
            for t in range(ET):
                nc.sync.dma_start(out=qt_sb[t][:], in_=qT[128 * t:128 * (t + 1), :])
                nc.sync.dma_start(out=kt_sb[t][:], in_=kT[128 * t:128 * (t + 1), :])
                nc.sync.dma_start(out=vt_sb[t][:], in_=vT[128 * t:128 * (t + 1), :])
                nc.sync.dma_start(out=wv_sb[t][:], in_=wv[128 * t:128 * (t + 1), :])
                nc.sync.dma_start(out=wq1_sb[t][:], in_=wq1[128 * t:128 * (t + 1), :])
                nc.sync.dma_start(out=wk2_sb[t][:], in_=wk2[128 * t:128 * (t + 1), :])
            bq1_sb = big.tile([HEADS, 1], F32)
            nc.sync.dma_start(out=bq1_sb[:], in_=bq1[:])
            bp_sb = big.tile([1, OUT], BF16)
            nc.sync.dma_start(out=bp_sb[:], in_=bp_r[:])
            ones_sb = big.tile([128, 128], BF16)
            nc.sync.dma_start(out=ones_sb[:], in_=ones128[:])
            eps_sb = big.tile([128, 1], F32)
            nc.any.memset(eps_sb[:], LN_EPS)
            neg1_sb = big.tile([128, 1], F32)
            nc.any.memset(neg1_sb[:], -1.0)
            if apply_gamma_beta:
                gma_sb = big.tile([128, OUT], F32)
                bta_sb = big.tile([128, OUT], F32)
                nc.sync.dma_start(out=gma_sb[:], in_=gma[:])
                nc.sync.dma_start(out=bta_sb[:], in_=bta[:])

            # ---- sq [16, IH], sk [16, L] --------------------------------
            p_sq = ps.tile([HEADS, IH], F32, tag="pBIG", bufs=2, name="p_sq")
            for t in range(ET):
                nc.tensor.matmul(p_sq[:], wq1_sb[t][:], qt_sb[t][:],
                                 start=(t == 0), stop=(t == ET - 1))
            sq_sb = big.tile([HEADS, IH], BF16)
            nc.vector.tensor_scalar_add(sq_sb[:], p_sq[:], bq1_sb[:])

            sk_sb = big.tile([HEADS, L], BF16)
            sk_f32 = big.tile([HEADS, L], F32)
            for half in range(2):
                p_sk = ps.tile([HEADS, 512], F32, tag="pBIG", bufs=2, name="p_sk")
                for t in range(ET):
                    nc.tensor.matmul(p_sk[:], wk2_sb[t][:],
                                     kt_sb[t][:, 512 * half:512 * (half + 1)],
                                     start=(t == 0), stop=(t == ET - 1))
                nc.vector.tensor_copy(sk_f32[:, 512 * half:512 * (half + 1)], p_sk[:])
            nc.vector.tensor_copy(sk_sb[:], sk_f32[:])

            # rank-2 outer-product operands, packed 4 heads per column range
            # on partition bases 0/32/64/96: head h -> base 32*(h%4),
            # column block h//4.  skz rows: (sk_h, ones); szr rows: (ones, sq_h)
            skz = big.tile([128, 4 * L], BF16)
            szr = big.tile([128, 4 * IH], BF16)
            for bs in (0, 32, 64, 96):
                nc.sync.dma_start(out=skz[bs + 1:bs + 2, :], in_=ones128[0:32, :])
                nc.sync.dma_start(out=szr[bs:bs + 1, :], in_=ones128[0:16, :])
            for h in range(HEADS):
                bs, cb = 32 * (h % 4), h // 4
                nc.sync.dma_start(out=skz[bs:bs + 1, L * cb:L * (cb + 1)],
                                  in_=sk_sb[h:h + 1, :])
                nc.sync.dma_start(out=szr[bs + 1:bs + 2, IH * cb:IH * (cb + 1)],
                                  in_=sq_sb[h:h + 1, :])

            # exp-path operands: w = exp(-2 sk), u = exp(-2 sq - 2 bq1),
            # packed like skz/szr.  z+1 = w (x) u + 1 (x) 1; tanh = (1-z)/(1+z)
            u_sb = big.tile([HEADS, IH], BF16)
            nbq1 = big.tile([HEADS, 1], F32)
            nc.vector.tensor_scalar_mul(nbq1[:], bq1_sb[:], -2.0)
            nc.scalar.activation(u_sb[:], p_sq[:], AF.Exp, scale=-2.0,
                                 bias=nbq1[:])
            w_sb = big.tile([HEADS, L], BF16)
            nc.scalar.activation(w_sb[:], sk_f32[:], AF.Exp, scale=-2.0)
            wz = big.tile([128, 4 * L], BF16)
            uz = big.tile([128, 4 * IH], BF16)
            for bs in (0, 32, 64, 96):
                nc.sync.dma_start(out=wz[bs + 1:bs + 2, :], in_=ones128[0:32, :])
                nc.sync.dma_start(out=uz[bs + 1:bs + 2, :], in_=ones128[0:16, :])
            for h in range(HEADS):
                bs, cb = 32 * (h % 4), h // 4
                nc.sync.dma_start(out=wz[bs:bs + 1, L * cb:L * (cb + 1)],
                                  in_=w_sb[h:h + 1, :])
                nc.sync.dma_start(out=uz[bs:bs + 1, IH * cb:IH * (cb + 1)],
                                  in_=u_sb[h:h + 1, :])

            # ---- vh_aug [j, 16*65]: per head 64 cols + ones col ---------
            vh_sb = [big.tile([128, HEADS * (HDIM + 1)], BF16, tag="vh8",
                              bufs=8, name=f"vh{t}") for t in range(JT)]

            def emit_vh():
                for jt in range(JT):
                    ones_cols = vh_sb[jt][:, :].rearrange(
                        "p (h c) -> p h c", c=HDIM + 1)[:, :, HDIM:HDIM + 1]
                    nc.vector.tensor_copy(ones_cols, ones_sb[:, 0:HEADS])
                for jt in range(JT):
                    p_vh = ps.tile([128, OUT], F32, tag="pBIG", bufs=2,
                                   name="p_vh")
                    for half in range(2):
                        sl = slice(512 * half, 512 * (half + 1))
                        for t in range(ET):
                            nc.tensor.matmul(
                                p_vh[:, sl], vt_sb[t][:, 128 * jt:128 * (jt + 1)],
                                wv_sb[t][:, sl], start=(t == 0),
                                stop=(t == ET - 1))
                    # scatter 16 head-chunks of 64 into the 65-strided layout
                    dst = vh_sb[jt][:, :].rearrange(
                        "p (h c) -> p h c", c=HDIM + 1)[:, :, 0:HDIM]
                    src = p_vh[:, :].rearrange("p (h c) -> p h c", c=HDIM)
                    nc.vector.tensor_copy(dst, src)

            # ---- attention + context, one head-pair at a time -----------
            ctx_sb = [big.tile([128, IH], BF16, tag="v8", bufs=8,
                               name=f"ctx{t}") for t in range(JT)]

            def normalize(p_ctx, pair, h):
                """ctx rows 0:64 /= row 64; write into ctx_sb[pair] rows."""
                den = work.tile([128, IH], BF16, tag="den", bufs=2, name="den")
                nc.vector.tensor_copy(den[64:65, :], p_ctx[64:65, :])
                p_db = ps.tile([64, IH], F32, tag="pC", bufs=4, name="p_db")
                nc.tensor.matmul(p_db[:], ones_sb[64:65, 0:64], den[64:65, :],
                                 start=True, stop=True)
                rb = work.tile([64, IH], F32, tag="rb", bufs=2, name="rb")
                nc.vector.reciprocal_approx_fast(out=rb[:], in_=p_db[:])
                if h % 2 == 0:
                    nc.vector.tensor_tensor(ctx_sb[pair][0:64, :], p_ctx[0:64, :],
                                            rb[:], ALU.mult)
                else:
                    bnc = work.tile([64, IH], BF16, tag="bnc", bufs=2, name="bnc")
                    nc.vector.tensor_tensor(bnc[:], p_ctx[0:64, :], rb[:],
                                            ALU.mult)
                    nc.gpsimd.dma_start(out=ctx_sb[pair][64:128, :], in_=bnc[:])

            N_TANH_PAIRS = 3     # pairs 0..2 tanh-path, rest exp/recip-path

            def produce_g(pair):
                tanh_path = pair < N_TANH_PAIRS
                hA, hB = 2 * pair, 2 * pair + 1
                g_tiles = []
                for jt in range(JT):
                    p_s = ps.tile([128, 2 * IH], F32, tag="pBIG", bufs=2,
                                  name="p_s")
                    for idx, h in enumerate((hA, hB)):
                        bs, cb = 32 * (h % 4), h // 4
                        lt, rt = (skz, szr) if tanh_path else (wz, uz)
                        nc.tensor.matmul(
                            p_s[:, IH * idx:IH * (idx + 1)],
                            lt[bs:bs + 2, L * cb + 128 * jt:L * cb + 128 * (jt + 1)],
                            rt[bs:bs + 2, IH * cb:IH * (cb + 1)],
                            start=True, stop=True,
                            tile_position=(bs, 0) if bs == 96 else None)
                    g = big.tile([128, 2 * IH], BF16, tag="gp", bufs=16,
                                 name="g")
                    if tanh_path:
                        th = big.tile([128, 2 * IH], F32, tag="thp", bufs=4,
                                      name="th")
                        nc.scalar.activation(th[:], p_s[:], AF.Tanh)
                        nc.scalar.activation(g[:], th[:], AF.Exp)
                    else:
                        # p_s holds z+1; tanh = 2/(1+z) - 1, g = exp(tanh)
                        rc = big.tile([128, 2 * IH], F32, tag="thp", bufs=4,
                                      name="rc")
                        nc.vector.reciprocal_approx_fast(out=rc[:], in_=p_s[:])
                        nc.scalar.activation(g[:], rc[:], AF.Exp, scale=2.0,
                                             bias=neg1_sb[:])
                    g_tiles.append(g)
                return g_tiles

            def consume_g(pair, g_tiles):
                hA, hB = 2 * pair, 2 * pair + 1
                p_ctxA = ps.tile([65, IH], F32, tag="pC", bufs=4, name="p_ctxA")
                p_ctxB = ps.tile([65, IH], F32, tag="pC", bufs=4, name="p_ctxB")
                for jt in range(JT):
                    g = g_tiles[jt]
                    for idx, (h, p_ctx) in enumerate(((hA, p_ctxA),
                                                      (hB, p_ctxB))):
                        nc.tensor.matmul(
                            p_ctx[:],
                            vh_sb[jt][:, (HDIM + 1) * h:(HDIM + 1) * (h + 1)],
                            g[:, IH * idx:IH * (idx + 1)],
                            start=(jt == 0), stop=(jt == JT - 1))
                normalize(p_ctxA, pair, hA)
                normalize(p_ctxB, pair, hB)

            g01 = [produce_g(p) for p in (0, 1)]
            emit_vh()
            for p in (0, 1):
                consume_g(p, g01[p])
            for pair in range(2, HEADS // 2):
                consume_g(pair, produce_g(pair))

            # ---- output projection + bias + LayerNorm -------------------
            wp_sb = [big.tile([128, OUT], BF16, tag="w8", bufs=8,
                              name=f"wp{t}") for t in range(JT)]
            for t in range(JT):
                nc.sync.dma_start(out=wp_sb[t][:], in_=wp[128 * t:128 * (t + 1), :])

            for mi in range(IH // 128):
                p_out = ps.tile([128, OUT], F32, tag="pBIG", bufs=2, name="p_out")
                for half in range(2):
                    sl = slice(512 * half, 512 * (half + 1))
                    for t in range(JT):
                        nc.tensor.matmul(p_out[:, sl],
                                         ctx_sb[t][:, 128 * mi:128 * (mi + 1)],
                                         wp_sb[t][:, sl], start=(t == 0),
                                         stop=False)
                    nc.tensor.matmul(p_out[:, sl], ones_sb[0:1, :],
                                     bp_sb[:, sl], start=False, stop=True)
                # LayerNorm over the 1024 free elems of each row
                stats = work.tile([128, 12], F32, tag="st", bufs=2, name="stats")
                aggr = work.tile([128, 2], F32, tag="ag", bufs=2, name="aggr")
                for half in range(2):
                    nc.vector.bn_stats(stats[:, 6 * half:6 * (half + 1)],
                                       p_out[:, 512 * half:512 * (half + 1)])
                nc.vector.bn_aggr(aggr[:], stats[:])
                std = work.tile([128, 1], F32, tag="sd", bufs=2, name="std")
                nc.scalar.activation(std[:], aggr[:, 1:2], AF.Sqrt,
                                     bias=eps_sb[:])
                rstd = work.tile([128, 1], F32, tag="rs", bufs=2, name="rstd")
                nc.vector.reciprocal(rstd[:], std[:])
                nmr = work.tile([128, 1], F32, tag="nm", bufs=2, name="nmr")
                nc.vector.tensor_tensor(nmr[:], aggr[:, 0:1], rstd[:], ALU.mult)
                nc.vector.tensor_scalar_mul(nmr[:], nmr[:], -1.0)
                y = big.tile([128, OUT], F32, tag="wv8", bufs=8, name="y")
                nc.scalar.activation(y[:], p_out[:], AF.Identity,
                                     scale=rstd[:], bias=nmr[:])
                if apply_gamma_beta:
                    nc.vector.tensor_tensor(y[:], y[:], gma_sb[:], ALU.mult)
                    nc.vector.tensor_tensor(y[:], y[:], bta_sb[:], ALU.add)
                nc.sync.dma_start(out=out_d[128 * mi:128 * (mi + 1), :], in_=y[:])

    nc.compile()
    return nc


def kernel(k, q, v, Wq, bq, Wk, Wv, Wp, bp, attn_w, gamma, beta):
    k = np.asarray(k, np.float32)
    q = np.asarray(q, np.float32)
    v = np.asarray(v, np.float32)
    w1 = np.asarray(attn_w, np.float64)[:HDIM]
    w2 = np.asarray(attn_w, np.float64)[HDIM:]
    Wq1 = (np.asarray(Wq, np.float64).reshape(E, HEADS, HDIM) @ w1).astype(np.float32)
    Wk2 = (np.asarray(Wk, np.float64).reshape(E, HEADS, HDIM) @ w2).astype(np.float32)
    bq1 = (np.asarray(bq, np.float64).reshape(HEADS, HDIM) @ w1).astype(np.float32)

    gamma = np.asarray(gamma, np.float32)
    beta = np.asarray(beta, np.float32)
    apply_gb = not (np.all(gamma == 1.0) and np.all(beta == 0.0))

    if apply_gb not in _CACHE:
        _CACHE[apply_gb] = _build(apply_gb)
    nc = _CACHE[apply_gb]

    shared = {
        "wv": np.ascontiguousarray(Wv).astype(BF),
        "wp": np.ascontiguousarray(Wp).astype(BF),
        "wq1": Wq1.astype(BF),
        "wk2": Wk2.astype(BF),
        "bq1": bq1.reshape(HEADS, 1),
        "bp_r": np.ascontiguousarray(bp, np.float32).reshape(1, OUT).astype(BF),
        "ones128": np.ones((128, 128), BF),
    }
    if apply_gb:
        shared["gma"] = np.ascontiguousarray(
            np.broadcast_to(gamma[None, :], (128, OUT)), np.float32)
        shared["bta"] = np.ascontiguousarray(
            np.broadcast_to(beta[None, :], (128, OUT)), np.float32)

    in_maps = []
    for c in range(N_CORES):
        b, ih = c // 2, c % 2
        in_maps.append({
            "qT": np.ascontiguousarray(q[b, IH * ih:IH * (ih + 1), :].T).astype(BF),
            "kT": np.ascontiguousarray(k[b].T).astype(BF),
            "vT": np.ascontiguousarray(v[b].T).astype(BF),
            **shared,
        })

    global _LAST_IN_MAPS
    _LAST_IN_MAPS = in_maps
    res = run_bass_kernel_spmd(nc, in_maps, list(range(N_CORES)), trace=False)
    out = np.empty((B, L, OUT), np.float32)
    for c in range(N_CORES):
        b, ih = c // 2, c % 2
        out[b, IH * ih:IH * (ih + 1), :] = res.results[c]["out"]
    return out


# revision 14
# speedup vs baseline: 1.0201x; 1.0201x over previous
"""Trainium2 Bass kernel for MultiHeadAttention with MLP (additive) scores.

Reference computation (per batch b):
  qh = q @ Wq + bq; kh = k @ Wk; vh = v @ Wv           (heads x hdim = 16 x 64)
  s[i,j,n] = qh[i,n,:]@w1 + kh[j,n,:]@w2               (folded: q @ Wq1 + bq1, k @ Wk2)
  attn = softmax_j(tanh(s)); ctx = attn @ vh
  out = LayerNorm(ctx @ Wp + bp) * gamma + beta

Sharding: core c = 2*b + ihalf handles batch b = c//2, query rows
[ihalf*512, ihalf*512+512), all 16 heads. No collectives.

On-chip schedule per core (all big matmuls in float32r, 1 cyc/row):
  sq[16,512] = Wq1^T @ qT (+bq1), sk[16,1024] = Wk2^T @ kT
  vh[j, h*65:(h+1)*65] = [v @ Wv heads | ones-col]     (denominator trick)
  per head-pair: s = sk (x) 1 + 1 (x) sq   (rank-2 PE outer into PSUM pair)
                 t = Tanh(s); g = Exp(t)               (ACT, same table set)
                 ctx_h[65,512] += vh_aug[jt]^T @ g     (row 64 = softmax denom)
  normalize: rb = 1/(ones (x) den); ctx *= rb          (DVE recip-approx + TT)
  out[i,:] = ctx_all^T @ Wp + 1 (x) bp; LayerNorm on DVE/ACT; DMA out.
"""
import ml_dtypes
import numpy as np

BF = ml_dtypes.bfloat16

import concourse.bacc as bacc
import concourse.mybir as mybir
from concourse.tile import TileContext
from concourse.bass_utils import run_bass_kernel_spmd

F32 = mybir.dt.float32
BF16 = mybir.dt.bfloat16
AF = mybir.ActivationFunctionType
ALU = mybir.AluOpType

B, L, E, HEADS, HDIM, OUT = 4, 1024, 1024, 16, 64, 1024
N_CORES = 8
IH = L // 2          # 512 query rows per core
JT = L // 128        # 8 key tiles
ET = E // 128        # 8 embed tiles
LN_EPS = 1e-6

_CACHE = {}


def _build(apply_gamma_beta: bool):
    nc = bacc.Bacc("TRN2", target_bir_lowering=False, debug=False,
                   num_devices=N_CORES)

    def din(name, shape, dt=BF16):
        return nc.dram_tensor(name, shape, dt, kind="ExternalInput").ap()

    qT = din("qT", [E, IH])          # q-half transposed  [e, i]
    kT = din("kT", [E, L])           # [e, j]
    vT = din("vT", [E, L])           # [e, j]
    wv = din("wv", [E, OUT])         # [e, nd]
    wp = din("wp", [HEADS * HDIM, OUT])   # [nd, o]
    wq1 = din("wq1", [E, HEADS])
    wk2 = din("wk2", [E, HEADS])
    bq1 = din("bq1", [HEADS, 1], F32)
    bp_r = din("bp_r", [1, OUT])
    ones128 = din("ones128", [128, 128])
    if apply_gamma_beta:
        gma = din("gma", [128, OUT], F32)
        bta = din("bta", [128, OUT], F32)
    out_d = nc.dram_tensor("out", [IH, OUT], F32, kind="ExternalOutput").ap()

    with TileContext(nc) as tc:
        with (
            tc.tile_pool(name="big", bufs=1) as big,       # long-lived SBUF
            tc.tile_pool(name="work", bufs=1) as work,     # pipelined SBUF
            tc.tile_pool(name="ps", bufs=1, space="PSUM") as ps,
        ):
            # ---- static loads -------------------------------------------
            kt_sb = [big.tile([128, L], BF16, tag="w8", bufs=8, name=f"kt{t}")
                     for t in range(ET)]
            vt_sb = [big.tile([128, L], BF16, tag="v8", bufs=8, name=f"vt{t}")
                     for t in range(ET)]
            wv_sb = [big.tile([128, OUT], BF16, tag="wv8", bufs=8, name=f"wv{t}")
                     for t in range(ET)]
            qt_sb = [big.tile([128, IH], BF16, tag="q8", bufs=8, name=f"qt{t}")
                     for t in range(ET)]
            wq1_sb = [big.tile([128, HEADS], BF16, tag="wq1", bufs=8, name=f"wq1{t}")
                      for t in range(ET)]
            wk2_sb = [big.tile([128, HEADS], BF16, tag="wk2", bufs=8, name=f"wk2{t}")
                      for t in range(ET)]
            for t in range(ET):
                nc.sync.dma_start(out=kt_sb[t][:], in_=kT[128 * t:128 * (t + 1), :])
                nc.sync.dma_start(out=wk2_sb[t][:], in_=wk2[128 * t:128 * (t + 1), :])
                nc.sync.dma_start(out=qt_sb[t][:], in_=qT[128 * t:128 * (t + 1), :])
                nc.sync.dma_start(out=wq1_sb[t][:], in_=wq1[128 * t:128 * (t + 1), :])
            for t in range(ET):
                nc.sync.dma_start(out=vt_sb[t][:], in_=vT[128 * t:128 * (t + 1), :])
                nc.sync.dma_start(out=wv_sb[t][:], in_=wv[128 * t:128 * (t + 1), :])
            bq1_sb = big.tile([HEADS, 1], F32)
            nc.sync.dma_start(out=bq1_sb[:], in_=bq1[:])
            bp_sb = big.tile([1, OUT], BF16)
            nc.sync.dma_start(out=bp_sb[:], in_=bp_r[:])
            ones_sb = big.tile([128, 128], BF16)
            nc.sync.dma_start(out=ones_sb[:], in_=ones128[:])
            eps_sb = big.tile([128, 1], F32)
            nc.any.memset(eps_sb[:], LN_EPS)
            neg1_sb = big.tile([128, 1], F32)
            nc.any.memset(neg1_sb[:], -1.0)
            if apply_gamma_beta:
                gma_sb = big.tile([128, OUT], F32)
                bta_sb = big.tile([128, OUT], F32)
                nc.sync.dma_start(out=gma_sb[:], in_=gma[:])
                nc.sync.dma_start(out=bta_sb[:], in_=bta[:])

            # ---- sq [16, IH], sk [16, L] --------------------------------
            p_sq = ps.tile([HEADS, IH], F32, tag="pBIG", bufs=2, name="p_sq")
            for t in range(ET):
                nc.tensor.matmul(p_sq[:], wq1_sb[t][:], qt_sb[t][:],
                                 start=(t == 0), stop=(t == ET - 1))
            sq_sb = big.tile([HEADS, IH], BF16)
            nc.vector.tensor_scalar_add(sq_sb[:], p_sq[:], bq1_sb[:])

            sk_sb = big.tile([HEADS, L], BF16)
            sk_f32 = big.tile([HEADS, L], F32)
            for half in range(2):
                p_sk = ps.tile([HEADS, 512], F32, tag="pBIG", bufs=2, name="p_sk")
                for t in range(ET):
                    nc.tensor.matmul(p_sk[:], wk2_sb[t][:],
                                     kt_sb[t][:, 512 * half:512 * (half + 1)],
                                     start=(t == 0), stop=(t == ET - 1))
                nc.vector.tensor_copy(sk_f32[:, 512 * half:512 * (half + 1)], p_sk[:])
            nc.vector.tensor_copy(sk_sb[:], sk_f32[:])

            # rank-2 outer-product operands, packed 4 heads per column range
            # on partition bases 0/32/64/96: head h -> base 32*(h%4),
            # column block h//4.  skz rows: (sk_h, ones); szr rows: (ones, sq_h)
            skz = big.tile([128, 4 * L], BF16)
            szr = big.tile([128, 4 * IH], BF16)
            for bs in (0, 32, 64, 96):
                nc.sync.dma_start(out=skz[bs + 1:bs + 2, :], in_=ones128[0:32, :])
                nc.sync.dma_start(out=szr[bs:bs + 1, :], in_=ones128[0:16, :])
            for h in range(HEADS):
                bs, cb = 32 * (h % 4), h // 4
                nc.sync.dma_start(out=skz[bs:bs + 1, L * cb:L * (cb + 1)],
                                  in_=sk_sb[h:h + 1, :])
                nc.sync.dma_start(out=szr[bs + 1:bs + 2, IH * cb:IH * (cb + 1)],
                                  in_=sq_sb[h:h + 1, :])

            # exp-path operands: w = exp(-2 sk), u = exp(-2 sq - 2 bq1),
            # packed like skz/szr.  z+1 = w (x) u + 1 (x) 1; tanh = (1-z)/(1+z)
            u_sb = big.tile([HEADS, IH], BF16)
            nbq1 = big.tile([HEADS, 1], F32)
            nc.vector.tensor_scalar_mul(nbq1[:], bq1_sb[:], -2.0)
            nc.scalar.activation(u_sb[:], p_sq[:], AF.Exp, scale=-2.0,
                                 bias=nbq1[:])
            w_sb = big.tile([HEADS, L], BF16)
            nc.scalar.activation(w_sb[:], sk_f32[:], AF.Exp, scale=-2.0)
            wz = big.tile([128, 4 * L], BF16)
            uz = big.tile([128, 4 * IH], BF16)
            for bs in (0, 32, 64, 96):
                nc.sync.dma_start(out=wz[bs + 1:bs + 2, :], in_=ones128[0:32, :])
                nc.sync.dma_start(out=uz[bs + 1:bs + 2, :], in_=ones128[0:16, :])
            for h in range(HEADS):
                bs, cb = 32 * (h % 4), h // 4
                nc.sync.dma_start(out=wz[bs:bs + 1, L * cb:L * (cb + 1)],
                                  in_=w_sb[h:h + 1, :])
                nc.sync.dma_start(out=uz[bs:bs + 1, IH * cb:IH * (cb + 1)],
                                  in_=u_sb[h:h + 1, :])

            # ---- vh_aug [j, 16*65]: per head 64 cols + ones col ---------
            vh_sb = [big.tile([128, HEADS * (HDIM + 1)], BF16, tag="vh8",
                              bufs=8, name=f"vh{t}") for t in range(JT)]

            def emit_vh():
                for jt in range(JT):
                    ones_cols = vh_sb[jt][:, :].rearrange(
                        "p (h c) -> p h c", c=HDIM + 1)[:, :, HDIM:HDIM + 1]
                    nc.vector.tensor_copy(ones_cols, ones_sb[:, 0:HEADS])
                for jt in range(JT):
                    p_vh = ps.tile([128, OUT], F32, tag="pBIG", bufs=2,
                                   name="p_vh")
                    for half in range(2):
                        sl = slice(512 * half, 512 * (half + 1))
                        for t in range(ET):
                            nc.tensor.matmul(
                                p_vh[:, sl], vt_sb[t][:, 128 * jt:128 * (jt + 1)],
                                wv_sb[t][:, sl], start=(t == 0),
                                stop=(t == ET - 1))
                    # scatter 16 head-chunks of 64 into the 65-strided layout
                    dst = vh_sb[jt][:, :].rearrange(
                        "p (h c) -> p h c", c=HDIM + 1)[:, :, 0:HDIM]
                    src = p_vh[:, :].rearrange("p (h c) -> p h c", c=HDIM)
                    nc.vector.tensor_copy(dst, src)

            # ---- attention + context, one head-pair at a time -----------
            ctx_sb = [big.tile([128, IH], BF16, tag="v8", bufs=8,
                               name=f"ctx{t}") for t in range(JT)]

            def normalize(p_ctx, pair, h):
                """ctx rows 0:64 /= row 64; write into ctx_sb[pair] rows."""
                den = work.tile([128, IH], BF16, tag="den", bufs=2, name="den")
                nc.vector.tensor_copy(den[64:65, :], p_ctx[64:65, :])
                p_db = ps.tile([64, IH], F32, tag="pC", bufs=4, name="p_db")
                nc.tensor.matmul(p_db[:], ones_sb[64:65, 0:64], den[64:65, :],
                                 start=True, stop=True)
                rb = work.tile([64, IH], F32, tag="rb", bufs=2, name="rb")
                nc.vector.reciprocal_approx_fast(out=rb[:], in_=p_db[:])
                if h % 2 == 0:
                    nc.vector.tensor_tensor(ctx_sb[pair][0:64, :], p_ctx[0:64, :],
                                            rb[:], ALU.mult)
                else:
                    bnc = work.tile([64, IH], BF16, tag="bnc", bufs=2, name="bnc")
                    nc.vector.tensor_tensor(bnc[:], p_ctx[0:64, :], rb[:],
                                            ALU.mult)
                    nc.gpsimd.dma_start(out=ctx_sb[pair][64:128, :], in_=bnc[:])

            N_TANH_PAIRS = 3     # pairs 0..2 tanh-path, rest exp/recip-path

            def produce_g(pair):
                tanh_path = pair < N_TANH_PAIRS
                hA, hB = 2 * pair, 2 * pair + 1
                g_tiles = []
                for jt in range(JT):
                    p_s = ps.tile([128, 2 * IH], F32, tag="pBIG", bufs=2,
                                  name="p_s")
                    for idx, h in enumerate((hA, hB)):
                        bs, cb = 32 * (h % 4), h // 4
                        lt, rt = (skz, szr) if tanh_path else (wz, uz)
                        nc.tensor.matmul(
                            p_s[:, IH * idx:IH * (idx + 1)],
                            lt[bs:bs + 2, L * cb + 128 * jt:L * cb + 128 * (jt + 1)],
                            rt[bs:bs + 2, IH * cb:IH * (cb + 1)],
                            start=True, stop=True,
                            tile_position=(bs, 0) if bs == 96 else None)
                    g = big.tile([128, 2 * IH], BF16, tag="gp", bufs=24,
                                 name="g")
                    if tanh_path:
                        th = big.tile([128, 2 * IH], F32, tag="thp", bufs=4,
                                      name="th")
                        nc.scalar.activation(th[:], p_s[:], AF.Tanh)
                        nc.scalar.activation(g[:], th[:], AF.Exp)
                    else:
                        # p_s holds z+1; tanh = 2/(1+z) - 1, g = exp(tanh)
                        rc = big.tile([128, 2 * IH], F32, tag="thp", bufs=4,
                                      name="rc")
                        nc.vector.reciprocal_approx_fast(out=rc[:], in_=p_s[:])
                        nc.scalar.activation(g[:], rc[:], AF.Exp, scale=2.0,
                                             bias=neg1_sb[:])
                    g_tiles.append(g)
                return g_tiles

            def consume_g(pair, g_tiles):
                hA, hB = 2 * pair, 2 * pair + 1
                p_ctxA = ps.tile([65, IH], F32, tag="pC", bufs=4, name="p_ctxA")
                p_ctxB = ps.tile([65, IH], F32, tag="pC", bufs=4, name="p_ctxB")
                for jt in range(JT):
                    g = g_tiles[jt]
                    for idx, (h, p_ctx) in enumerate(((hA, p_ctxA),
                                                      (hB, p_ctxB))):
                        nc.tensor.matmul(
                            p_ctx[:],
                            vh_sb[jt][:, (HDIM + 1) * h:(HDIM + 1) * (h + 1)],
                            g[:, IH * idx:IH * (idx + 1)],
                            start=(jt == 0), stop=(jt == JT - 1))
                normalize(p_ctxA, pair, hA)
                normalize(p_ctxB, pair, hB)

            gq = {0: produce_g(0), 1: produce_g(1)}
            emit_vh()
            for pair in range(2, HEADS // 2):
                gq[pair] = produce_g(pair)
                consume_g(pair - 2, gq.pop(pair - 2))
            for pair in (HEADS // 2 - 2, HEADS // 2 - 1):
                consume_g(pair, gq.pop(pair))

            # ---- output projection + bias + LayerNorm -------------------
            wp_sb = [big.tile([128, OUT], BF16, tag="w8", bufs=8,
                              name=f"wp{t}") for t in range(JT)]
            for t in range(JT):
                nc.sync.dma_start(out=wp_sb[t][:], in_=wp[128 * t:128 * (t + 1), :])

            for mi in range(IH // 128):
                p_out = ps.tile([128, OUT], F32, tag="pBIG", bufs=2, name="p_out")
                for half in range(2):
                    sl = slice(512 * half, 512 * (half + 1))
                    for t in range(JT):
                        nc.tensor.matmul(p_out[:, sl],
                                         ctx_sb[t][:, 128 * mi:128 * (mi + 1)],
                                         wp_sb[t][:, sl], start=(t == 0),
                                         stop=False)
                    nc.tensor.matmul(p_out[:, sl], ones_sb[0:1, :],
                                     bp_sb[:, sl], start=False, stop=True)
                # LayerNorm over the 1024 free elems of each row
                stats = work.tile([128, 12], F32, tag="st", bufs=2, name="stats")
                aggr = work.tile([128, 2], F32, tag="ag", bufs=2, name="aggr")
                for half in range(2):
                    nc.vector.bn_stats(stats[:, 6 * half:6 * (half + 1)],
                                       p_out[:, 512 * half:512 * (half + 1)])
                nc.vector.bn_aggr(aggr[:], stats[:])
                std = work.tile([128, 1], F32, tag="sd", bufs=2, name="std")
                nc.scalar.activation(std[:], aggr[:, 1:2], AF.Sqrt,
                                     bias=eps_sb[:])
                rstd = work.tile([128, 1], F32, tag="rs", bufs=2, name="rstd")
                nc.vector.reciprocal(rstd[:], std[:])
                nmr = work.tile([128, 1], F32, tag="nm", bufs=2, name="nmr")
                nc.vector.tensor_tensor(nmr[:], aggr[:, 0:1], rstd[:], ALU.mult)
                nc.vector.tensor_scalar_mul(nmr[:], nmr[:], -1.0)
                y = big.tile([128, OUT], F32, tag="wv8", bufs=8, name="y")
                nc.scalar.activation(y[:], p_out[:], AF.Identity,
                                     scale=rstd[:], bias=nmr[:])
                if apply_gamma_beta:
                    nc.vector.tensor_tensor(y[:], y[:], gma_sb[:], ALU.mult)
                    nc.vector.tensor_tensor(y[:], y[:], bta_sb[:], ALU.add)
                nc.sync.dma_start(out=out_d[128 * mi:128 * (mi + 1), :], in_=y[:])

    nc.compile()
    return nc


def kernel(k, q, v, Wq, bq, Wk, Wv, Wp, bp, attn_w, gamma, beta):
    k = np.asarray(k, np.float32)
    q = np.asarray(q, np.float32)
    v = np.asarray(v, np.float32)
    w1 = np.asarray(attn_w, np.float64)[:HDIM]
    w2 = np.asarray(attn_w, np.float64)[HDIM:]
    Wq1 = (np.asarray(Wq, np.float64).reshape(E, HEADS, HDIM) @ w1).astype(np.float32)
    Wk2 = (np.asarray(Wk, np.float64).reshape(E, HEADS, HDIM) @ w2).astype(np.float32)
    bq1 = (np.asarray(bq, np.float64).reshape(HEADS, HDIM) @ w1).astype(np.float32)

    gamma = np.asarray(gamma, np.float32)
    beta = np.asarray(beta, np.float32)
    apply_gb = not (np.all(gamma == 1.0) and np.all(beta == 0.0))

    if apply_gb not in _CACHE:
        _CACHE[apply_gb] = _build(apply_gb)
    nc = _CACHE[apply_gb]

    shared = {
        "wv": np.ascontiguousarray(Wv).astype(BF),
        "wp": np.ascontiguousarray(Wp).astype(BF),
        "wq1": Wq1.astype(BF),
        "wk2": Wk2.astype(BF),
        "bq1": bq1.reshape(HEADS, 1),
        "bp_r": np.ascontiguousarray(bp, np.float32).reshape(1, OUT).astype(BF),
        "ones128": np.ones((128, 128), BF),
    }
    if apply_gb:
        shared["gma"] = np.ascontiguousarray(
            np.broadcast_to(gamma[None, :], (128, OUT)), np.float32)
        shared["bta"] = np.ascontiguousarray(
            np.broadcast_to(beta[None, :], (128, OUT)), np.float32)

    in_maps = []
    for c in range(N_CORES):
        b, ih = c // 2, c % 2
        in_maps.append({
            "qT": np.ascontiguousarray(q[b, IH * ih:IH * (ih + 1), :].T).astype(BF),
            "kT": np.ascontiguousarray(k[b].T).astype(BF),
            "vT": np.ascontiguousarray(v[b].T).astype(BF),
            **shared,
        })

    global _LAST_IN_MAPS
    _LAST_IN_MAPS = in_maps
    res = run_bass_kernel_spmd(nc, in_maps, list(range(N_CORES)), trace=False)
    out = np.empty((B, L, OUT), np.float32)
    for c in range(N_CORES):
        b, ih = c // 2, c % 2
        out[b, IH * ih:IH * (ih + 1), :] = res.results[c]["out"]
    return out


# revision 15
# speedup vs baseline: 1.1031x; 1.0813x over previous
"""Trainium2 Bass kernel for MultiHeadAttention with MLP (additive) scores.

Reference computation (per batch b):
  qh = q @ Wq + bq; kh = k @ Wk; vh = v @ Wv           (heads x hdim = 16 x 64)
  s[i,j,n] = qh[i,n,:]@w1 + kh[j,n,:]@w2               (folded: q @ Wq1 + bq1, k @ Wk2)
  attn = softmax_j(tanh(s)); ctx = attn @ vh
  out = LayerNorm(ctx @ Wp + bp) * gamma + beta

Sharding: core c = 2*b + ihalf handles batch b = c//2, query rows
[ihalf*512, ihalf*512+512), all 16 heads. No collectives.

On-chip schedule per core (all big matmuls in float32r, 1 cyc/row):
  sq[16,512] = Wq1^T @ qT (+bq1), sk[16,1024] = Wk2^T @ kT
  vh[j, h*65:(h+1)*65] = [v @ Wv heads | ones-col]     (denominator trick)
  per head-pair: s = sk (x) 1 + 1 (x) sq   (rank-2 PE outer into PSUM pair)
                 t = Tanh(s); g = Exp(t)               (ACT, same table set)
                 ctx_h[65,512] += vh_aug[jt]^T @ g     (row 64 = softmax denom)
  normalize: rb = 1/(ones (x) den); ctx *= rb          (DVE recip-approx + TT)
  out[i,:] = ctx_all^T @ Wp + 1 (x) bp; LayerNorm on DVE/ACT; DMA out.
"""
import ml_dtypes
import numpy as np

BF = ml_dtypes.bfloat16

import concourse.bacc as bacc
import concourse.mybir as mybir
from concourse.tile import TileContext
from concourse.bass_utils import run_bass_kernel_spmd

F32 = mybir.dt.float32
BF16 = mybir.dt.bfloat16
AF = mybir.ActivationFunctionType
ALU = mybir.AluOpType

B, L, E, HEADS, HDIM, OUT = 4, 1024, 1024, 16, 64, 1024
N_CORES = 8
IH = L // 2          # 512 query rows per core
JT = L // 128        # 8 key tiles
ET = E // 128        # 8 embed tiles
LN_EPS = 1e-6

_CACHE = {}


def _build(apply_gamma_beta: bool):
    nc = bacc.Bacc("TRN2", target_bir_lowering=False, debug=False,
                   num_devices=N_CORES)

    def din(name, shape, dt=BF16):
        return nc.dram_tensor(name, shape, dt, kind="ExternalInput").ap()

    qT = din("qT", [E, IH])          # q-half transposed  [e, i]
    kT = din("kT", [E, L])           # [e, j]
    vT = din("vT", [E, L])           # [e, j]
    wv = din("wv", [E, OUT])         # [e, nd]
    wp = din("wp", [HEADS * HDIM, OUT])   # [nd, o]
    wq1 = din("wq1", [E, HEADS])
    wk2 = din("wk2", [E, HEADS])
    bq1 = din("bq1", [HEADS, 1], F32)
    bp_r = din("bp_r", [1, OUT])
    ones128 = din("ones128", [128, 128])
    if apply_gamma_beta:
        gma = din("gma", [128, OUT], F32)
        bta = din("bta", [128, OUT], F32)
    out_d = nc.dram_tensor("out", [IH, OUT], F32, kind="ExternalOutput").ap()

    with TileContext(nc) as tc:
        with (
            tc.tile_pool(name="big", bufs=1) as big,       # long-lived SBUF
            tc.tile_pool(name="work", bufs=1) as work,     # pipelined SBUF
            tc.tile_pool(name="ps", bufs=1, space="PSUM") as ps,
        ):
            # ---- static loads: one big DMA per tensor (full HBM BW) -----
            kt_all = big.tile([128, ET * L], BF16, tag="kt", bufs=1)
            wk2_all = big.tile([128, ET * HEADS], BF16, tag="wk2", bufs=1)
            qt_all = big.tile([128, ET * IH], BF16, tag="q8", bufs=1)
            wq1_all = big.tile([128, ET * HEADS], BF16, tag="wq1", bufs=1)
            vt_all = big.tile([128, ET * L], BF16, tag="vt", bufs=1)
            wv_all = big.tile([128, ET * OUT], BF16, tag="wv8", bufs=1)
            nc.sync.dma_start(out=kt_all[:],
                              in_=kT[:].rearrange("(t p) n -> p t n", p=128))
            nc.sync.dma_start(out=wk2_all[:],
                              in_=wk2[:].rearrange("(t p) n -> p t n", p=128))
            nc.sync.dma_start(out=qt_all[:],
                              in_=qT[:].rearrange("(t p) n -> p t n", p=128))
            nc.sync.dma_start(out=wq1_all[:],
                              in_=wq1[:].rearrange("(t p) n -> p t n", p=128))
            nc.sync.dma_start(out=vt_all[:],
                              in_=vT[:].rearrange("(t p) n -> p t n", p=128))
            nc.sync.dma_start(out=wv_all[:],
                              in_=wv[:].rearrange("(t p) n -> p t n", p=128))
            kt_sb = [kt_all[:, L * t:L * (t + 1)] for t in range(ET)]
            vt_sb = [vt_all[:, L * t:L * (t + 1)] for t in range(ET)]
            wv_sb = [wv_all[:, OUT * t:OUT * (t + 1)] for t in range(ET)]
            qt_sb = [qt_all[:, IH * t:IH * (t + 1)] for t in range(ET)]
            wq1_sb = [wq1_all[:, HEADS * t:HEADS * (t + 1)] for t in range(ET)]
            wk2_sb = [wk2_all[:, HEADS * t:HEADS * (t + 1)] for t in range(ET)]
            bq1_sb = big.tile([HEADS, 1], F32)
            nc.sync.dma_start(out=bq1_sb[:], in_=bq1[:])
            bp_sb = big.tile([1, OUT], BF16)
            nc.sync.dma_start(out=bp_sb[:], in_=bp_r[:])
            ones_sb = big.tile([128, 128], BF16)
            nc.sync.dma_start(out=ones_sb[:], in_=ones128[:])
            eps_sb = big.tile([128, 1], F32)
            nc.any.memset(eps_sb[:], LN_EPS)
            neg1_sb = big.tile([128, 1], F32)
            nc.any.memset(neg1_sb[:], -1.0)
            if apply_gamma_beta:
                gma_sb = big.tile([128, OUT], F32)
                bta_sb = big.tile([128, OUT], F32)
                nc.sync.dma_start(out=gma_sb[:], in_=gma[:])
                nc.sync.dma_start(out=bta_sb[:], in_=bta[:])

            # ---- sq [16, IH], sk [16, L] --------------------------------
            p_sq = ps.tile([HEADS, IH], F32, tag="pBIG", bufs=2, name="p_sq")
            for t in range(ET):
                nc.tensor.matmul(p_sq[:], wq1_sb[t], qt_sb[t],
                                 start=(t == 0), stop=(t == ET - 1))
            sq_sb = big.tile([HEADS, IH], BF16)
            nc.vector.tensor_scalar_add(sq_sb[:], p_sq[:], bq1_sb[:])

            sk_sb = big.tile([HEADS, L], BF16)
            sk_f32 = big.tile([HEADS, L], F32)
            for half in range(2):
                p_sk = ps.tile([HEADS, 512], F32, tag="pBIG", bufs=2, name="p_sk")
                for t in range(ET):
                    nc.tensor.matmul(p_sk[:], wk2_sb[t],
                                     kt_sb[t][:, 512 * half:512 * (half + 1)],
                                     start=(t == 0), stop=(t == ET - 1))
                nc.vector.tensor_copy(sk_f32[:, 512 * half:512 * (half + 1)], p_sk[:])
            nc.vector.tensor_copy(sk_sb[:], sk_f32[:])

            # rank-2 outer-product operands, packed 4 heads per column range
            # on partition bases 0/32/64/96: head h -> base 32*(h%4),
            # column block h//4.  skz rows: (sk_h, ones); szr rows: (ones, sq_h)
            skz = big.tile([128, 4 * L], BF16)
            szr = big.tile([128, 4 * IH], BF16)
            for bs in (0, 32, 64, 96):
                nc.sync.dma_start(out=skz[bs + 1:bs + 2, :], in_=ones128[0:32, :])
                nc.sync.dma_start(out=szr[bs:bs + 1, :], in_=ones128[0:16, :])
            for h in range(HEADS):
                bs, cb = 32 * (h % 4), h // 4
                nc.sync.dma_start(out=skz[bs:bs + 1, L * cb:L * (cb + 1)],
                                  in_=sk_sb[h:h + 1, :])
                nc.sync.dma_start(out=szr[bs + 1:bs + 2, IH * cb:IH * (cb + 1)],
                                  in_=sq_sb[h:h + 1, :])

            # exp-path operands: w = exp(-2 sk), u = exp(-2 sq - 2 bq1),
            # packed like skz/szr.  z+1 = w (x) u + 1 (x) 1; tanh = (1-z)/(1+z)
            u_sb = big.tile([HEADS, IH], BF16)
            nbq1 = big.tile([HEADS, 1], F32)
            nc.vector.tensor_scalar_mul(nbq1[:], bq1_sb[:], -2.0)
            nc.scalar.activation(u_sb[:], p_sq[:], AF.Exp, scale=-2.0,
                                 bias=nbq1[:])
            w_sb = big.tile([HEADS, L], BF16)
            nc.scalar.activation(w_sb[:], sk_f32[:], AF.Exp, scale=-2.0)
            wz = big.tile([128, 4 * L], BF16)
            uz = big.tile([128, 4 * IH], BF16)
            for bs in (0, 32, 64, 96):
                nc.sync.dma_start(out=wz[bs + 1:bs + 2, :], in_=ones128[0:32, :])
                nc.sync.dma_start(out=uz[bs + 1:bs + 2, :], in_=ones128[0:16, :])
            for h in range(HEADS):
                bs, cb = 32 * (h % 4), h // 4
                nc.sync.dma_start(out=wz[bs:bs + 1, L * cb:L * (cb + 1)],
                                  in_=w_sb[h:h + 1, :])
                nc.sync.dma_start(out=uz[bs:bs + 1, IH * cb:IH * (cb + 1)],
                                  in_=u_sb[h:h + 1, :])

            # ---- vh_aug [j, 16*65]: per head 64 cols + ones col ---------
            vh_sb = [big.tile([128, HEADS * (HDIM + 1)], BF16, tag="vh8",
                              bufs=8, name=f"vh{t}") for t in range(JT)]

            def emit_vh():
                for jt in range(JT):
                    ones_cols = vh_sb[jt][:, :].rearrange(
                        "p (h c) -> p h c", c=HDIM + 1)[:, :, HDIM:HDIM + 1]
                    nc.vector.tensor_copy(ones_cols, ones_sb[:, 0:HEADS])
                for jt in range(JT):
                    p_vh = ps.tile([128, OUT], F32, tag="pBIG", bufs=2,
                                   name="p_vh")
                    for half in range(2):
                        sl = slice(512 * half, 512 * (half + 1))
                        for t in range(ET):
                            nc.tensor.matmul(
                                p_vh[:, sl], vt_sb[t][:, 128 * jt:128 * (jt + 1)],
                                wv_sb[t][:, sl], start=(t == 0),
                                stop=(t == ET - 1))
                    # scatter 16 head-chunks of 64 into the 65-strided layout
                    dst = vh_sb[jt][:, :].rearrange(
                        "p (h c) -> p h c", c=HDIM + 1)[:, :, 0:HDIM]
                    src = p_vh[:, :].rearrange("p (h c) -> p h c", c=HDIM)
                    nc.vector.tensor_copy(dst, src)

            # ---- attention + context, one head-pair at a time -----------
            ctx_sb = [big.tile([128, IH], BF16, tag="ctx", bufs=8,
                               name=f"ctx{t}") for t in range(JT)]

            def normalize(p_ctx, pair, h):
                """ctx rows 0:64 /= row 64; write into ctx_sb[pair] rows."""
                den = work.tile([128, IH], BF16, tag="den", bufs=2, name="den")
                nc.vector.tensor_copy(den[64:65, :], p_ctx[64:65, :])
                p_db = ps.tile([64, IH], F32, tag="pC", bufs=4, name="p_db")
                nc.tensor.matmul(p_db[:], ones_sb[64:65, 0:64], den[64:65, :],
                                 start=True, stop=True)
                rb = work.tile([64, IH], F32, tag="rb", bufs=2, name="rb")
                nc.vector.reciprocal_approx_fast(out=rb[:], in_=p_db[:])
                if h % 2 == 0:
                    nc.vector.tensor_tensor(ctx_sb[pair][0:64, :], p_ctx[0:64, :],
                                            rb[:], ALU.mult)
                else:
                    bnc = work.tile([64, IH], BF16, tag="bnc", bufs=2, name="bnc")
                    nc.vector.tensor_tensor(bnc[:], p_ctx[0:64, :], rb[:],
                                            ALU.mult)
                    nc.gpsimd.dma_start(out=ctx_sb[pair][64:128, :], in_=bnc[:])

            N_TANH_PAIRS = 3     # pairs 0..2 tanh-path, rest exp/recip-path

            def produce_g(pair):
                tanh_path = pair < N_TANH_PAIRS
                hA, hB = 2 * pair, 2 * pair + 1
                g_tiles = []
                for jt in range(JT):
                    p_s = ps.tile([128, 2 * IH], F32, tag="pBIG", bufs=2,
                                  name="p_s")
                    for idx, h in enumerate((hA, hB)):
                        bs, cb = 32 * (h % 4), h // 4
                        lt, rt = (skz, szr) if tanh_path else (wz, uz)
                        nc.tensor.matmul(
                            p_s[:, IH * idx:IH * (idx + 1)],
                            lt[bs:bs + 2, L * cb + 128 * jt:L * cb + 128 * (jt + 1)],
                            rt[bs:bs + 2, IH * cb:IH * (cb + 1)],
                            start=True, stop=True,
                            tile_position=(bs, 0) if bs == 96 else None)
                    g = big.tile([128, 2 * IH], BF16, tag="gp", bufs=24,
                                 name="g")
                    if tanh_path:
                        th = big.tile([128, 2 * IH], F32, tag="thp", bufs=4,
                                      name="th")
                        nc.scalar.activation(th[:], p_s[:], AF.Tanh)
                        nc.scalar.activation(g[:], th[:], AF.Exp)
                    else:
                        # p_s holds z+1; tanh = 2/(1+z) - 1, g = exp(tanh)
                        rc = big.tile([128, 2 * IH], F32, tag="thp", bufs=4,
                                      name="rc")
                        nc.vector.reciprocal_approx_fast(out=rc[:], in_=p_s[:])
                        nc.scalar.activation(g[:], rc[:], AF.Exp, scale=2.0,
                                             bias=neg1_sb[:])
                    g_tiles.append(g)
                return g_tiles

            def consume_g(pair, g_tiles):
                hA, hB = 2 * pair, 2 * pair + 1
                p_ctxA = ps.tile([65, IH], F32, tag="pC", bufs=4, name="p_ctxA")
                p_ctxB = ps.tile([65, IH], F32, tag="pC", bufs=4, name="p_ctxB")
                for jt in range(JT):
                    g = g_tiles[jt]
                    for idx, (h, p_ctx) in enumerate(((hA, p_ctxA),
                                                      (hB, p_ctxB))):
                        nc.tensor.matmul(
                            p_ctx[:],
                            vh_sb[jt][:, (HDIM + 1) * h:(HDIM + 1) * (h + 1)],
                            g[:, IH * idx:IH * (idx + 1)],
                            start=(jt == 0), stop=(jt == JT - 1))
                normalize(p_ctxA, pair, hA)
                normalize(p_ctxB, pair, hB)

            gq = {0: produce_g(0), 1: produce_g(1)}
            emit_vh()
            for pair in range(2, HEADS // 2):
                gq[pair] = produce_g(pair)
                consume_g(pair - 2, gq.pop(pair - 2))
            for pair in (HEADS // 2 - 2, HEADS // 2 - 1):
                consume_g(pair, gq.pop(pair))

            # ---- output projection + bias + LayerNorm -------------------
            wp_all = big.tile([128, JT * OUT], BF16, tag="kt", bufs=1)
            nc.sync.dma_start(out=wp_all[:],
                              in_=wp[:].rearrange("(t p) n -> p t n", p=128))
            wp_sb = [wp_all[:, OUT * t:OUT * (t + 1)] for t in range(JT)]

            for mi in range(IH // 128):
                p_out = ps.tile([128, OUT], F32, tag="pBIG", bufs=2, name="p_out")
                for half in range(2):
                    sl = slice(512 * half, 512 * (half + 1))
                    for t in range(JT):
                        nc.tensor.matmul(p_out[:, sl],
                                         ctx_sb[t][:, 128 * mi:128 * (mi + 1)],
                                         wp_sb[t][:, sl], start=(t == 0),
                                         stop=False)
                    nc.tensor.matmul(p_out[:, sl], ones_sb[0:1, :],
                                     bp_sb[:, sl], start=False, stop=True)
                # LayerNorm over the 1024 free elems of each row
                stats = work.tile([128, 12], F32, tag="st", bufs=2, name="stats")
                aggr = work.tile([128, 2], F32, tag="ag", bufs=2, name="aggr")
                for half in range(2):
                    nc.vector.bn_stats(stats[:, 6 * half:6 * (half + 1)],
                                       p_out[:, 512 * half:512 * (half + 1)])
                nc.vector.bn_aggr(aggr[:], stats[:])
                std = work.tile([128, 1], F32, tag="sd", bufs=2, name="std")
                nc.scalar.activation(std[:], aggr[:, 1:2], AF.Sqrt,
                                     bias=eps_sb[:])
                rstd = work.tile([128, 1], F32, tag="rs", bufs=2, name="rstd")
                nc.vector.reciprocal(rstd[:], std[:])
                nmr = work.tile([128, 1], F32, tag="nm", bufs=2, name="nmr")
                nc.vector.tensor_tensor(nmr[:], aggr[:, 0:1], rstd[:], ALU.mult)
                nc.vector.tensor_scalar_mul(nmr[:], nmr[:], -1.0)
                y = big.tile([128, OUT], F32, tag="thp", bufs=4, name="y")
                nc.scalar.activation(y[:], p_out[:], AF.Identity,
                                     scale=rstd[:], bias=nmr[:])
                if apply_gamma_beta:
                    nc.vector.tensor_tensor(y[:], y[:], gma_sb[:], ALU.mult)
                    nc.vector.tensor_tensor(y[:], y[:], bta_sb[:], ALU.add)
                nc.sync.dma_start(out=out_d[128 * mi:128 * (mi + 1), :], in_=y[:])

    nc.compile()
    return nc


def kernel(k, q, v, Wq, bq, Wk, Wv, Wp, bp, attn_w, gamma, beta):
    k = np.asarray(k, np.float32)
    q = np.asarray(q, np.float32)
    v = np.asarray(v, np.float32)
    w1 = np.asarray(attn_w, np.float64)[:HDIM]
    w2 = np.asarray(attn_w, np.float64)[HDIM:]
    Wq1 = (np.asarray(Wq, np.float64).reshape(E, HEADS, HDIM) @ w1).astype(np.float32)
    Wk2 = (np.asarray(Wk, np.float64).reshape(E, HEADS, HDIM) @ w2).astype(np.float32)
    bq1 = (np.asarray(bq, np.float64).reshape(HEADS, HDIM) @ w1).astype(np.float32)

    gamma = np.asarray(gamma, np.float32)
    beta = np.asarray(beta, np.float32)
    apply_gb = not (np.all(gamma == 1.0) and np.all(beta == 0.0))

    if apply_gb not in _CACHE:
        _CACHE[apply_gb] = _build(apply_gb)
    nc = _CACHE[apply_gb]

    shared = {
        "wv": np.ascontiguousarray(Wv).astype(BF),
        "wp": np.ascontiguousarray(Wp).astype(BF),
        "wq1": Wq1.astype(BF),
        "wk2": Wk2.astype(BF),
        "bq1": bq1.reshape(HEADS, 1),
        "bp_r": np.ascontiguousarray(bp, np.float32).reshape(1, OUT).astype(BF),
        "ones128": np.ones((128, 128), BF),
    }
    if apply_gb:
        shared["gma"] = np.ascontiguousarray(
            np.broadcast_to(gamma[None, :], (128, OUT)), np.float32)
        shared["bta"] = np.ascontiguousarray(
            np.broadcast_to(beta[None, :], (128, OUT)), np.float32)

    in_maps = []
    for c in range(N_CORES):
        b, ih = c // 2, c % 2
        in_maps.append({
            "qT": np.ascontiguousarray(q[b, IH * ih:IH * (ih + 1), :].T).astype(BF),
            "kT": np.ascontiguousarray(k[b].T).astype(BF),
            "vT": np.ascontiguousarray(v[b].T).astype(BF),
            **shared,
        })

    global _LAST_IN_MAPS
    _LAST_IN_MAPS = in_maps
    res = run_bass_kernel_spmd(nc, in_maps, list(range(N_CORES)), trace=False)
    out = np.empty((B, L, OUT), np.float32)
    for c in range(N_CORES):
        b, ih = c // 2, c % 2
        out[b, IH * ih:IH * (ih + 1), :] = res.results[c]["out"]
    return out
